# revision 9
# baseline (speedup 1.0000x reference)
"""Class-balanced cross-entropy loss kernel for Trainium2 (8 NeuronCores).

Problem: output [4,8,64,128,128] f32 logits, labels [4,1,64,128,128] int
(values 0..7).  loss = mean over present classes of (per-class mean CE).

Strategy (data-parallel over the flattened voxel axis, 524288 voxels/core):
  per-voxel CE loss  l_i = logsumexp_c(x_ic) - x_i[lab_i]
  per-class sums     sums[c]  = S_lse[c] - S_g[c]
     S_lse[c] = sum_{i: lab=c} lse_i      (masked accumulate, DVE)
     S_g[c]   = sum_{i: lab=c} x_i[c]     (masked accumulate, DVE)
     counts[c]                            (masked accumulate, DVE)
  final scalar combined on host from tiny per-core partials.

Device layout per core: 4 superblocks of 8 slabs (slab = H*W = 16384 vox).
  x tiles  [128, 4096] bf16, two per superblock (class halves):
     xlo[chat*32+v1, shat*512+v2] = x[b, chat,   d, v1, v2]   chat in 0..3
     xhi[...]                     = x[b, chat+4, d, v1, v2]
  exp on ACT; s = sum over 8 classes via two PE matmuls (G32 stationary
  group-sum matrix, second matmul accumulates with start=False) -> PSUM.
  lse = log(s) on ACT -> per-core [128, 4096] bf16 buffer.
  Masked per-class accumulations via scalar_tensor_tensor / tensor_scalar
  with fused per-partition accum_out (bf16 operands -> 2x/4x DVE modes).
"""

import numpy as np
import ml_dtypes

import concourse.bass as bass
import concourse.bacc as bacc
import concourse.mybir as mybir
from concourse import bass_utils, tile

BF16 = mybir.dt.bfloat16
F32 = mybir.dt.float32
NPBF16 = ml_dtypes.bfloat16

N_CORES = 8
B, C, D, H, W = 4, 8, 64, 128, 128
N_SB = 4                                # superblocks per core
SB_COLS = 4096
VOX_PER_CORE = 32 * H * W               # 524288

_PROG_CACHE = {}


def _build_program():
    nc = bacc.Bacc("TRN2", target_bir_lowering=False, debug=False)

    x_in = nc.dram_tensor("x", [N_SB, 2, 128, SB_COLS], BF16, kind="ExternalInput")
    lr_in = nc.dram_tensor("labrep", [N_SB, 128, SB_COLS], BF16, kind="ExternalInput")
    ll_in = nc.dram_tensor("lablse", [128, SB_COLS], BF16, kind="ExternalInput")
    g32_in = nc.dram_tensor("g32", [128, 32], BF16, kind="ExternalInput")
    pm4_in = nc.dram_tensor("pm4", [128, 2], F32, kind="ExternalInput")
    out_d = nc.dram_tensor("partials", [3, 128, 8], F32, kind="ExternalOutput")

    with tile.TileContext(nc) as tc:
        with (
            tc.tile_pool(name="const", bufs=1) as cpool,
            tc.tile_pool(name="io", bufs=3) as iopool,
            tc.tile_pool(name="work", bufs=2) as wpool,
            tc.tile_pool(name="psum", bufs=4, space="PSUM") as ppool,
        ):
            g32 = cpool.tile([128, 32], BF16)
            nc.sync.dma_start(g32[:], g32_in[:])
            pm4 = cpool.tile([128, 2], F32)
            nc.sync.dma_start(pm4[:], pm4_in[:])
            lab_lse = cpool.tile([128, SB_COLS], BF16)
            nc.sync.dma_start(lab_lse[:], ll_in[:])

            lse = cpool.tile([128, SB_COLS], BF16)
            # tiny DVE reads that absorb DMA semaphore waits so the
            # wait-slot-limited TensorScalarPtr ops need at most one wait
            dummy = cpool.tile([128, 4], F32)
            nc.vector.tensor_copy(dummy[:, 0:2], pm4[:])
            nc.vector.tensor_copy(dummy[:, 2:3], lab_lse[:, 0:1])
            sg_acc = cpool.tile([128, 8], F32)
            slse_acc = cpool.tile([128, 8], F32)
            cnt_acc = cpool.tile([128, 8], F32)

            for sb in range(N_SB):
                xlo = iopool.tile([128, SB_COLS], BF16, tag="xlo")
                xhi = iopool.tile([128, SB_COLS], BF16, tag="xhi")
                lr_sb = iopool.tile([128, SB_COLS], BF16, tag="lr")
                nc.sync.dma_start(xlo[:], x_in[sb, 0])
                nc.sync.dma_start(xhi[:], x_in[sb, 1])
                nc.sync.dma_start(lr_sb[:], lr_in[sb])

                # absorb the lr DMA wait on the DVE
                nc.vector.tensor_copy(dummy[:, 3:4], lr_sb[:, 0:1])

                # S_g partials: out = (labrep == class(p)) * x, accum over free
                for h, x_sb in ((0, xlo), (1, xhi)):
                    sc = wpool.tile([128, SB_COLS], BF16, tag="sc")
                    nc.vector.scalar_tensor_tensor(
                        sc[:],
                        lr_sb[:],
                        pm4[:, h : h + 1],
                        x_sb[:],
                        mybir.AluOpType.is_equal,
                        mybir.AluOpType.mult,
                        accum_out=sg_acc[:, 2 * sb + h : 2 * sb + h + 1],
                    )

                # softmax denominator: exp on ACT, class-group sums on PE
                elo = wpool.tile([128, SB_COLS], BF16, tag="elo")
                ehi = wpool.tile([128, SB_COLS], BF16, tag="ehi")
                nc.scalar.activation(elo[:], xlo[:], mybir.ActivationFunctionType.Exp)
                nc.scalar.activation(ehi[:], xhi[:], mybir.ActivationFunctionType.Exp)

                for g in range(2):  # two psum tiles of 4 slabs each
                    ps = ppool.tile([128, 512], F32, tag="ps")
                    for q in range(4):
                        sl = 512 * (4 * g + q)
                        nc.tensor.matmul(
                            ps[32 * q : 32 * (q + 1), :],
                            g32[:],
                            elo[:, sl : sl + 512],
                            start=True,
                            stop=False,
                            tile_position=(0, 32 * q),
                        )
                        nc.tensor.matmul(
                            ps[32 * q : 32 * (q + 1), :],
                            g32[:],
                            ehi[:, sl : sl + 512],
                            start=False,
                            stop=True,
                            tile_position=(0, 32 * q),
                        )
                    u = 2 * sb + g
                    nc.scalar.activation(
                        lse[:, 512 * u : 512 * (u + 1)],
                        ps[:],
                        mybir.ActivationFunctionType.Ln,
                    )

            # per-class masked sums of lse and counts (whole core at once)
            for c in range(8):
                sc2 = wpool.tile([128, SB_COLS], BF16, tag="sc2")
                nc.vector.scalar_tensor_tensor(
                    sc2[:],
                    lab_lse[:],
                    float(c),
                    lse[:],
                    mybir.AluOpType.is_equal,
                    mybir.AluOpType.mult,
                    accum_out=slse_acc[:, c : c + 1],
                )
                sc3 = wpool.tile([128, SB_COLS], BF16, tag="sc3")
                nc.vector.tensor_scalar(
                    sc3[:],
                    lab_lse[:],
                    float(c),
                    None,
                    mybir.AluOpType.is_equal,
                    op1=mybir.AluOpType.add,
                    accum_out=cnt_acc[:, c : c + 1],
                )

            nc.sync.dma_start(out_d[0], sg_acc[:])
            nc.sync.dma_start(out_d[1], slse_acc[:])
            nc.sync.dma_start(out_d[2], cnt_acc[:])

    nc.compile()
    return nc


def _host_prep(output, labels):
    """Build per-core input maps (sharding + layout prep, no math)."""
    x = np.asarray(output)
    lab = np.asarray(labels).astype(np.int32)

    g32 = np.zeros((128, 32), dtype=NPBF16)
    for ch in range(4):
        for v1 in range(32):
            g32[ch * 32 + v1, v1] = 1.0
    pcls = np.arange(128, dtype=np.int32) // 32
    pm4 = np.stack([pcls, pcls + 4], axis=1).astype(np.float32)

    in_maps = []
    for k in range(N_CORES):
        b, d0 = k // 2, 32 * (k % 2)
        # [8c, 4sb, 8shat, 32v1, 512v2] -> [sb, chat, v1, shat, v2]
        xc = x[b, :, d0 : d0 + 32].reshape(8, 4, 8, 32, 512)
        xt = xc.transpose(1, 0, 3, 2, 4).astype(NPBF16)  # [sb, c, v1, shat, v2]
        x_prep = np.stack(
            [
                np.ascontiguousarray(xt[:, :4]).reshape(4, 128, 4096),
                np.ascontiguousarray(xt[:, 4:]).reshape(4, 128, 4096),
            ],
            axis=1,
        )

        lc = lab[b, 0, d0 : d0 + 32].reshape(4, 8, 32, 512).astype(NPBF16)
        # labrep[sb, chat*32+v1, shat*512+v2]
        lr = lc.transpose(0, 2, 1, 3).reshape(4, 1, 32, 4096)
        lr = np.ascontiguousarray(
            np.broadcast_to(lr, (4, 4, 32, 4096))
        ).reshape(4, 128, 4096)
        # lablse[(shat%4)*32+v1, (2*sb + shat//4)*512+v2]
        l2 = lc.reshape(4, 2, 4, 32, 512)  # [sb, sh, sl, v1, v2]
        ll = np.ascontiguousarray(l2.transpose(2, 3, 0, 1, 4)).reshape(128, 4096)

        in_maps.append(
            {"x": x_prep, "labrep": lr, "lablse": ll, "g32": g32, "pm4": pm4}
        )
    return in_maps


def _combine(results):
    """Host gather: reduce per-core [3,128,8] partials to the final scalar."""
    S_g = np.zeros(8, dtype=np.float64)
    S_lse = np.zeros(8, dtype=np.float64)
    cnt = np.zeros(8, dtype=np.float64)
    pclass = np.arange(128) // 32  # 0..3 per partition
    for r in results:
        p = np.asarray(r["partials"], dtype=np.float64)
        for ch in range(4):
            rows = pclass == ch
            S_g[ch] += p[0][rows, 0::2].sum()      # even cols = lo half
            S_g[ch + 4] += p[0][rows, 1::2].sum()  # odd cols = hi half
        S_lse += p[1].sum(axis=0)
        cnt += p[2].sum(axis=0)
    sums = S_lse - S_g
    present = cnt > 0
    class_means = sums / np.maximum(cnt, 1.0)
    n_valid = present.sum()
    loss = np.where(present, class_means, 0.0).sum() / n_valid
    return np.float32(loss)


def run(inputs_maps=None, trace=False, **inputs):
    if "nc" not in _PROG_CACHE:
        _PROG_CACHE["nc"] = _build_program()
    nc = _PROG_CACHE["nc"]
    in_maps = inputs_maps if inputs_maps is not None else _host_prep(**inputs)
    res = bass_utils.run_bass_kernel_spmd(
        nc, in_maps, list(range(N_CORES)), trace=trace
    )
    return res


def kernel(output, labels):
    res = run(output=output, labels=labels)
    return _combine(res.results)


# revision 13
# speedup vs baseline: 1.2265x; 1.2265x over previous
"""Class-balanced cross-entropy loss kernel for Trainium2 (8 NeuronCores).

Problem: output [4,8,64,128,128] f32 logits, labels [4,1,64,128,128] int
(values 0..7).  loss = mean over present classes of (per-class mean CE).

Strategy (data-parallel over the flattened voxel axis, 524288 voxels/core):
  per-voxel CE loss  l_i = logsumexp_c(x_ic) - x_i[lab_i]
  per-class sums     sums[c]  = S_lse[c] - S_g[c]
     S_lse[c] = sum_{i: lab=c} lse_i      (masked accumulate, DVE)
     S_g[c]   = sum_{i: lab=c} x_i[c]     (masked accumulate, DVE)
     counts[c]                            (masked accumulate, DVE)
  final scalar combined on host from tiny per-core partials.

Device layout per core: 4 superblocks of 8 slabs (slab = H*W = 16384 vox).
  x tiles  [128, 4096] bf16, two per superblock (class halves):
     xlo[chat*32+v1, shat*512+v2] = x[b, chat,   d, v1, v2]   chat in 0..3
     xhi[...]                     = x[b, chat+4, d, v1, v2]
  exp on ACT; s = sum over 8 classes via two PE matmuls (G32 stationary
  group-sum matrix, second matmul accumulates with start=False) -> PSUM.
  lse = log(s) on ACT -> per-core [128, 4096] bf16 buffer.
  Masked per-class accumulations via scalar_tensor_tensor / tensor_scalar
  with fused per-partition accum_out (bf16 operands -> 2x/4x DVE modes).
"""

import numpy as np
import ml_dtypes

import concourse.bass as bass
import concourse.bacc as bacc
import concourse.mybir as mybir
from concourse import bass_utils, tile

BF16 = mybir.dt.bfloat16
F32 = mybir.dt.float32
NPBF16 = ml_dtypes.bfloat16

N_CORES = 8
B, C, D, H, W = 4, 8, 64, 128, 128
N_SB = 4                                # superblocks per core
SB_COLS = 4096
VOX_PER_CORE = 32 * H * W               # 524288

_PROG_CACHE = {}


def _build_program():
    nc = bacc.Bacc("TRN2", target_bir_lowering=False, debug=False)

    x_in = nc.dram_tensor("x", [N_SB, 2, 128, SB_COLS], BF16, kind="ExternalInput")
    lr_in = nc.dram_tensor("labrep", [N_SB, 128, SB_COLS], BF16, kind="ExternalInput")
    ll_in = nc.dram_tensor("lablse", [128, SB_COLS], BF16, kind="ExternalInput")
    g32_in = nc.dram_tensor("g32", [128, 32], BF16, kind="ExternalInput")
    pm4_in = nc.dram_tensor("pm4", [128, 2], F32, kind="ExternalInput")
    hbias_in = nc.dram_tensor("hbias", [128, 7], F32, kind="ExternalInput")
    out_d = nc.dram_tensor("partials", [128, 47], F32, kind="ExternalOutput")

    with tile.TileContext(nc) as tc:
        with (
            tc.tile_pool(name="const", bufs=1) as cpool,
            tc.tile_pool(name="io", bufs=3) as iopool,
            tc.tile_pool(name="work", bufs=2) as wpool,
            tc.tile_pool(name="psum", bufs=4, space="PSUM") as ppool,
        ):
            g32 = cpool.tile([128, 32], BF16)
            nc.sync.dma_start(g32[:], g32_in[:])
            pm4 = cpool.tile([128, 2], F32)
            hbias = cpool.tile([128, 7], F32)
            nc.sync.dma_start(hbias[:], hbias_in[:])
            nc.sync.dma_start(pm4[:], pm4_in[:])
            lab_lse = cpool.tile([128, SB_COLS], BF16)
            nc.sync.dma_start(lab_lse[:], ll_in[:])

            lse = cpool.tile([128, SB_COLS], BF16)
            # tiny DVE reads that absorb DMA semaphore waits so the
            # wait-slot-limited TensorScalarPtr ops need at most one wait
            dummy = cpool.tile([128, 4], F32)
            nc.vector.tensor_copy(dummy[:, 0:2], pm4[:])
            nc.vector.tensor_copy(dummy[:, 2:3], lab_lse[:, 0:1])
            nc.scalar.activation(dummy[:, 3:4], hbias[:, 0:1], mybir.ActivationFunctionType.Copy)
            sg_acc = cpool.tile([128, 8], F32)
            slse_acc = cpool.tile([128, 32], F32)
            cnt_acc = cpool.tile([128, 7], F32)

            # counts via 7 tanh-staircase functionals of the labels on ACT:
            # m_j = sum_i tanh(8*(lab_i - j + 0.5)), j = 1..7; with the known
            # total this is a well-conditioned 8x8 system whose solution
            # rounds to the exact histogram (host side, f64).
            for j in range(7):
                sc3 = wpool.tile([128, SB_COLS], BF16, tag="sc3")
                nc.scalar.activation(
                    sc3[:],
                    lab_lse[:],
                    mybir.ActivationFunctionType.Tanh,
                    bias=hbias[:, j : j + 1],
                    scale=8.0,
                    accum_out=cnt_acc[:, j : j + 1],
                )

            for sb in range(N_SB):
                xlo = iopool.tile([128, SB_COLS], BF16, tag="xlo")
                xhi = iopool.tile([128, SB_COLS], BF16, tag="xhi")
                lr_sb = iopool.tile([128, SB_COLS], BF16, tag="lr")
                nc.sync.dma_start(xlo[:], x_in[sb, 0])
                nc.sync.dma_start(xhi[:], x_in[sb, 1])
                nc.sync.dma_start(lr_sb[:], lr_in[sb])

                # absorb the lr DMA wait on the DVE
                nc.vector.tensor_copy(dummy[:, 3:4], lr_sb[:, 0:1])

                # S_g partials: out = (labrep == class(p)) * x, accum over free
                for h, x_sb in ((0, xlo), (1, xhi)):
                    sc = wpool.tile([128, SB_COLS], BF16, tag="sc")
                    nc.vector.scalar_tensor_tensor(
                        sc[:],
                        lr_sb[:],
                        pm4[:, h : h + 1],
                        x_sb[:],
                        mybir.AluOpType.is_equal,
                        mybir.AluOpType.mult,
                        accum_out=sg_acc[:, 2 * sb + h : 2 * sb + h + 1],
                    )

                # softmax denominator: exp on ACT, class-group sums on PE
                elo = wpool.tile([128, SB_COLS], BF16, tag="elo")
                ehi = wpool.tile([128, SB_COLS], BF16, tag="ehi")
                nc.scalar.activation(elo[:], xlo[:], mybir.ActivationFunctionType.Exp)
                nc.scalar.activation(ehi[:], xhi[:], mybir.ActivationFunctionType.Exp)

                for g in range(2):  # two psum tiles of 4 slabs each
                    ps = ppool.tile([128, 512], F32, tag="ps")
                    for q in range(4):
                        sl = 512 * (4 * g + q)
                        nc.tensor.matmul(
                            ps[32 * q : 32 * (q + 1), :],
                            g32[:],
                            elo[:, sl : sl + 512],
                            start=True,
                            stop=False,
                            tile_position=(0, 32 * q),
                        )
                        nc.tensor.matmul(
                            ps[32 * q : 32 * (q + 1), :],
                            g32[:],
                            ehi[:, sl : sl + 512],
                            start=False,
                            stop=True,
                            tile_position=(0, 32 * q),
                        )
                    u = 2 * sb + g
                    nc.scalar.activation(
                        lse[:, 512 * u : 512 * (u + 1)],
                        ps[:],
                        mybir.ActivationFunctionType.Ln,
                    )

                # per-class masked lse sums (7 classes; class 7 comes from a
                # global lse sum on ACT) + global sum, on this superblock's
                # fresh [128, 1024] lse slice
                lsl = lse[:, 1024 * sb : 1024 * (sb + 1)]
                lll = lab_lse[:, 1024 * sb : 1024 * (sb + 1)]
                for c in range(7):
                    sc2 = wpool.tile([128, 1024], BF16, tag="sc2")
                    nc.vector.scalar_tensor_tensor(
                        sc2[:],
                        lll,
                        float(c),
                        lsl,
                        mybir.AluOpType.is_equal,
                        mybir.AluOpType.mult,
                        accum_out=slse_acc[:, 8 * sb + c : 8 * sb + c + 1],
                    )
                gl = wpool.tile([128, 1024], BF16, tag="gl")
                nc.scalar.activation(
                    gl[:],
                    lsl,
                    mybir.ActivationFunctionType.Copy,
                    accum_out=slse_acc[:, 8 * sb + 7 : 8 * sb + 8],
                )

            nc.sync.dma_start(out_d[:, 0:8], sg_acc[:])
            nc.sync.dma_start(out_d[:, 8:40], slse_acc[:])
            nc.sync.dma_start(out_d[:, 40:47], cnt_acc[:])

    nc.compile()
    return nc


def _host_prep(output, labels):
    """Build per-core input maps (sharding + layout prep, no math)."""
    x = np.asarray(output)
    lab = np.asarray(labels).astype(np.int32)

    g32 = np.zeros((128, 32), dtype=NPBF16)
    for ch in range(4):
        for v1 in range(32):
            g32[ch * 32 + v1, v1] = 1.0
    pcls = np.arange(128, dtype=np.int32) // 32
    pm4 = np.stack([pcls, pcls + 4], axis=1).astype(np.float32)

    in_maps = []
    for k in range(N_CORES):
        b, d0 = k // 2, 32 * (k % 2)
        # [8c, 4sb, 8shat, 32v1, 512v2] -> [sb, chat, v1, shat, v2]
        xc = x[b, :, d0 : d0 + 32].reshape(8, 4, 8, 32, 512)
        xt = xc.transpose(1, 0, 3, 2, 4).astype(NPBF16)  # [sb, c, v1, shat, v2]
        x_prep = np.stack(
            [
                np.ascontiguousarray(xt[:, :4]).reshape(4, 128, 4096),
                np.ascontiguousarray(xt[:, 4:]).reshape(4, 128, 4096),
            ],
            axis=1,
        )

        lc = lab[b, 0, d0 : d0 + 32].reshape(4, 8, 32, 512).astype(NPBF16)
        # labrep[sb, chat*32+v1, shat*512+v2]
        lr = lc.transpose(0, 2, 1, 3).reshape(4, 1, 32, 4096)
        lr = np.ascontiguousarray(
            np.broadcast_to(lr, (4, 4, 32, 4096))
        ).reshape(4, 128, 4096)
        # lablse[(shat%4)*32+v1, (2*sb + shat//4)*512+v2]
        l2 = lc.reshape(4, 2, 4, 32, 512)  # [sb, sh, sl, v1, v2]
        ll = np.ascontiguousarray(l2.transpose(2, 3, 0, 1, 4)).reshape(128, 4096)

        in_maps.append(
            {
                "x": x_prep,
                "labrep": lr,
                "lablse": ll,
                "g32": g32,
                "pm4": pm4,
                "hbias": np.broadcast_to(
                    8.0 * (0.5 - np.arange(1, 8, dtype=np.float32)), (128, 7)
                ).copy(),
            }
        )
    return in_maps


def _combine(results):
    """Host gather: reduce per-core [3,128,8] partials to the final scalar."""
    S_g = np.zeros(8, dtype=np.float64)
    S_lse = np.zeros(8, dtype=np.float64)
    cnt = np.zeros(8, dtype=np.float64)
    pclass = np.arange(128) // 32  # 0..3 per partition
    m = np.zeros(7, dtype=np.float64)
    glse = 0.0
    n_total = 0
    for r in results:
        p = np.asarray(r["partials"], dtype=np.float64)
        sg, slse, cn = p[:, 0:8], p[:, 8:40], p[:, 40:47]
        for ch in range(4):
            rows = pclass == ch
            S_g[ch] += sg[rows, 0::2].sum()      # even cols = lo half
            S_g[ch + 4] += sg[rows, 1::2].sum()  # odd cols = hi half
        sl = slse.sum(axis=0).reshape(4, 8).sum(axis=0)
        S_lse[:7] += sl[:7]
        glse += sl[7]
        m += cn.sum(axis=0)
        n_total += VOX_PER_CORE
    S_lse[7] = glse - S_lse[:7].sum()
    # histogram from tanh-staircase functionals
    js = np.arange(1, 8, dtype=np.float64)
    A = np.vstack(
        [np.ones(8), np.tanh(8.0 * (np.arange(8)[None, :] - js[:, None] + 0.5))]
    )
    cnt[:] = np.round(np.linalg.solve(A, np.concatenate([[n_total], m])))
    sums = S_lse - S_g
    present = cnt > 0
    class_means = sums / np.maximum(cnt, 1.0)
    n_valid = present.sum()
    loss = np.where(present, class_means, 0.0).sum() / n_valid
    return np.float32(loss)


def run(inputs_maps=None, trace=False, **inputs):
    if "nc" not in _PROG_CACHE:
        _PROG_CACHE["nc"] = _build_program()
    nc = _PROG_CACHE["nc"]
    in_maps = inputs_maps if inputs_maps is not None else _host_prep(**inputs)
    res = bass_utils.run_bass_kernel_spmd(
        nc, in_maps, list(range(N_CORES)), trace=trace
    )
    return res


def kernel(output, labels):
    res = run(output=output, labels=labels)
    return _combine(res.results)
